# revision 29
# baseline (speedup 1.0000x reference)
"""Trainium2 Bass kernel for a 2-layer FC-LSTM (B=512, T=128, D=300, H=1024).

Strategy: model-parallel over the gate/hidden dimension. Each of the 8
cores owns 128 hidden units per layer (512 gate rows), keeps its weight
slices resident in SBUF (bf16), and computes gates in transposed layout
[gates, batch] so every matmul is M=128 x K=128 x N=512. Hidden states
live transposed h.T = [H, B], quantized to fp8-e4m3 for the per-step
Shared-output HBM AllGather (halves wire bytes; weights stay bf16 so
the mixed-dtype recurrent matmuls keep the error at ~4e-3). Cell state,
PSUM accumulation and the mean/decoder stay fp32.

Schedule (the collective latency is ~10-17us launch-to-fire, so both
per-step AllGathers need nearly a full period of slack):
- Loop rotation: iteration t runs layer-0 of step t first (W0h stop-
  phase -> cell0 -> AG0(t) launch, all inside the first ~12us of the
  block), then finishes layer-1 of step t-1 (W1x start-phase + W1h
  stop-phase + cell1 + AG1(t-1) launch). Every GEMM thus reads an
  AllGather result that is at least ~one period old.
- x-projections for step t+1 are hoisted into step t (the PSUM gate
  banks free up once step t's activations have read them).
- Queues: gpsimd = AG input DMAs + collectives + x prefetch (kept free
  of anything that waits on a CC); sync = h0 copy-backs; scalar = h1
  copy-backs, emitted one iteration after their launch so the queue
  never idles on a CC; k-outer gate-triple matmul order keeps chunk
  consumption behind the copy-back landing rate.
"""
import sys

sys.path.insert(0, "/opt/trn_rl_repo")

import os
import numpy as np

import concourse.bass as bass
import concourse.bacc as bacc
import concourse.mybir as mybir
from concourse import tile
from concourse.bass_utils import run_bass_kernel_spmd

B, T, D, H = 512, 128, 300, 1024
NCORES = 8
HL = H // NCORES          # 128 hidden units owned per core (per layer)
GL = 4 * HL               # 512 gate rows owned per core
DK = [128, 128, 44]       # D=300 split into K-chunks
KH = H // 128             # 8 K-chunks over the hidden dim

F32 = mybir.dt.float32
F32R = mybir.dt.float32r
BF16 = mybir.dt.bfloat16
F8 = mybir.dt.float8e4
AF = mybir.ActivationFunctionType
_NO_COLL = bool(os.environ.get("KERNEL_NO_COLL"))
ALU = mybir.AluOpType

# pytorch gate order in the packed weights: i, f, g, o (m index).
# Processing order: f first (so sigma(f) overlaps later gates' matmuls).
M_ORDER = [1, 0, 2, 3]


def _build(t_steps, t_total=None):
    t_total = t_total or t_steps
    nc = bacc.Bacc("TRN2", target_bir_lowering=False, debug=False, num_devices=NCORES)

    xT = nc.dram_tensor("xT", [t_total, D, B], BF16, kind="ExternalInput")
    w0x = nc.dram_tensor("w0x", [128, 3 * GL], BF16, kind="ExternalInput")
    w0h = nc.dram_tensor("w0h", [128, KH * GL], BF16, kind="ExternalInput")
    w1x = nc.dram_tensor("w1x", [128, KH * GL], BF16, kind="ExternalInput")
    w1h = nc.dram_tensor("w1h", [128, KH * GL], BF16, kind="ExternalInput")
    b0d = nc.dram_tensor("b0d", [HL, 4], F32, kind="ExternalInput")
    b1d = nc.dram_tensor("b1d", [HL, 4], F32, kind="ExternalInput")
    wdec = nc.dram_tensor("wdec", [HL, 1], F32R, kind="ExternalInput")
    out_p = nc.dram_tensor("out_p", [1, B], F32, kind="ExternalOutput")

    rg = [list(range(NCORES))]

    with tile.TileContext(nc) as tc:
        with (
            tc.tile_pool(name="wpool", bufs=1) as wp,
            tc.tile_pool(name="hpool", bufs=1) as hp,
            tc.tile_pool(name="xpool", bufs=3) as xp,
            tc.tile_pool(name="zpool", bufs=1) as zp,
            tc.tile_pool(name="cpool", bufs=2) as cp,
            tc.tile_pool(name="pp", bufs=1, space="PSUM") as pp,
            tc.tile_pool(name="dram", bufs=2, space="DRAM") as dp,
        ):
            w0x_s = wp.tile([128, 3 * GL], BF16, tag="w0x", name="w0x")
            nc.sync.dma_start(w0x_s[:], w0x.ap())
            w0h_s = wp.tile([128, KH * GL], BF16, tag="w0h", name="w0h")
            nc.sync.dma_start(w0h_s[:], w0h.ap())
            w1x_s = wp.tile([128, KH * GL], BF16, tag="w1x", name="w1x")
            nc.sync.dma_start(w1x_s[:], w1x.ap())
            w1h_s = wp.tile([128, KH * GL], BF16, tag="w1h", name="w1h")
            nc.sync.dma_start(w1h_s[:], w1h.ap())
            b0_s = wp.tile([HL, 4], F32, tag="b0", name="b0")
            nc.sync.dma_start(b0_s[:], b0d.ap())
            b1_s = wp.tile([HL, 4], F32, tag="b1", name="b1")
            nc.sync.dma_start(b1_s[:], b1d.ap())
            wdec_s = wp.tile([HL, 1], F32R, tag="wdec", name="wdec")
            nc.sync.dma_start(wdec_s[:], wdec.ap())

            def wx_lhsT(kc, m):
                kp = DK[kc]
                return w0x_s[0:kp, kc * GL + m * 128 : kc * GL + (m + 1) * 128]

            def wh_lhsT(w_s, k, m):
                return w_s[0:128, k * GL + m * 128 : k * GL + (m + 1) * 128]

            def load_xt(t):
                xt = xp.tile([128, 3 * B], BF16, tag="xt", name="xt")
                for kc in range(3):
                    nc.gpsimd.dma_start(
                        xt[0 : DK[kc], kc * B : (kc + 1) * B],
                        xT.ap()[t, kc * 128 : kc * 128 + DK[kc], :],
                    )
                return xt

            def emit_wx(ps, xt, t):
                """Hoistable x-projection for step t into gate banks ps."""
                for m in M_ORDER:
                    for kc in range(3):
                        nc.tensor.matmul(
                            ps[m][:],
                            wx_lhsT(kc, m),
                            xt[0 : DK[kc], kc * B : (kc + 1) * B],
                            start=(kc == 0),
                            stop=(t == 0 and kc == 2),
                        )

            def ag_launch(src_bf, tag):
                """Issue the h-slice AllGather: in-dma + CC on gpsimd.
                Returns the gathered [H, B] DRAM tile."""
                agi = dp.tile([HL, B], F8, tag=f"{tag}i", name=f"{tag}i")
                ago = dp.tile(
                    [H, B], F8, tag=f"{tag}o", name=f"{tag}o",
                    addr_space="Local" if _NO_COLL else "Shared",
                )
                nc.gpsimd.dma_start(agi[:], src_bf[:])
                if not _NO_COLL:
                    nc.gpsimd.collective_compute(
                        "AllGather", ALU.bypass, replica_groups=rg,
                        ins=[agi.opt()], outs=[ago.opt()], unique_tensors="Yes",
                    )
                else:
                    for _k in range(KH):
                        nc.gpsimd.dma_start(ago[_k * 128 : (_k + 1) * 128, :], agi[:])
                return ago

            def ag_copyback(ago, tag, q, q0=None):
                """Copy the gathered [H, B] into SBUF h.T layout on queue q;
                chunk 0 optionally on q0 (saves a cross-queue sem hop)."""
                hT = hp.tile([128, KH * B], F8, tag=f"{tag}T", name=f"{tag}T", bufs=2)
                for k in range(KH):
                    qq = q0 if (k == 0 and q0 is not None) else q
                    qq.dma_start(
                        hT[:, k * B : (k + 1) * B], ago[k * 128 : (k + 1) * 128, :]
                    )
                return hT

            c0_prev = c1_prev = None
            h0T = h1T = None
            acc = None

            def emit_cell1(s, ps1s, yo_out):
                """Finish layer-1 for step s: activations + cell + acc.
                Returns (yo, tc1) for the h1loc mul."""
                nonlocal c1_prev, acc
                yi = zp.tile([128, B], F32, tag="yi", name="yi")
                yf = zp.tile([128, B], F32, tag="yf", name="yf")
                yg = zp.tile([128, B], F32, tag="yg", name="yg")
                if s > 0:
                    nc.scalar.activation(yf[:], ps1s[1][:], AF.Sigmoid, bias=b1_s[:, 1:2])
                nc.scalar.activation(yi[:], ps1s[0][:], AF.Sigmoid, bias=b1_s[:, 0:1])
                nc.scalar.activation(yg[:], ps1s[2][:], AF.Tanh, bias=b1_s[:, 2:3])
                c1 = cp.tile([128, B], F32, tag="c1", name="c1")
                if s == 0:
                    nc.vector.tensor_mul(c1[:], yi[:], yg[:])
                else:
                    da = zp.tile([128, B], F32, tag="da", name="da")
                    db = zp.tile([128, B], F32, tag="db", name="db")
                    nc.vector.tensor_mul(da[:], yf[:], c1_prev[:])
                    nc.vector.tensor_mul(db[:], yi[:], yg[:])
                    nc.vector.tensor_add(c1[:], da[:], db[:])
                c1_prev = c1
                nc.scalar.activation(yo_out[:], ps1s[3][:], AF.Sigmoid, bias=b1_s[:, 3:4])
                tc1 = zp.tile([128, B], BF16, tag="tc1", name="tc1")
                nc.scalar.activation(tc1[:], c1[:], AF.Tanh)
                h1f = zp.tile([128, B], F32, tag="h1f", name="h1f")
                nc.vector.tensor_mul(h1f[:], yo_out[:], tc1[:])
                if s == 0:
                    acc = cp.tile([128, B], F32R, tag="acc", name="acc")
                    nc.vector.tensor_copy(acc[:], h1f[:])
                else:
                    acc_new = cp.tile([128, B], F32R, tag="acc", name="acc")
                    nc.vector.tensor_add(acc_new[:], acc[:], h1f[:])
                    acc = acc_new
                return tc1

            xt_cur = load_xt(0)
            xt_nxt = load_xt(1) if t_steps > 1 else None
            ps0 = [pp.tile([128, B], F32, tag=f"ps0{g}", name=f"ps0{g}") for g in range(4)]
            emit_wx(ps0, xt_cur, 0)
            ps1_prev = None
            ago1_pend = None
            h0T_prev = None

            # Loop rotation: iteration t runs layer-0 of step t, then FINISHES
            # layer-1 of step t-1 (W1h stop-phase + cell1 + its AllGather), so
            # cell0(t) lands early in the period and both collectives get a
            # full period of latency slack.
            for t in range(t_steps):
                xt_nxt2 = load_xt(t + 2) if t + 2 < t_steps else None

                # -- layer0 recurrent stop-phase; per-gate (f,i,g,o) so each
                #    activation starts as soon as its gate's GEMM stops (the
                #    h0T input is a full period old -- no copy-back pacing) --
                if t > 0:
                    for m in M_ORDER:
                        for k in range(KH):
                            nc.tensor.matmul(
                                ps0[m][:],
                                wh_lhsT(w0h_s, k, m),
                                h0T[:, k * B : (k + 1) * B],
                                start=False,
                                stop=(k == KH - 1),
                            )

                # -- layer0 cell --
                zi = zp.tile([128, B], F32, tag="zi", name="zi")
                zf = zp.tile([128, B], F32, tag="zf", name="zf")
                zg = zp.tile([128, B], F32, tag="zg", name="zg")
                zo = zp.tile([128, B], BF16, tag="zo", name="zo")
                if t > 0:
                    nc.scalar.activation(zf[:], ps0[1][:], AF.Sigmoid, bias=b0_s[:, 1:2])
                nc.scalar.activation(zi[:], ps0[0][:], AF.Sigmoid, bias=b0_s[:, 0:1])
                nc.scalar.activation(zg[:], ps0[2][:], AF.Tanh, bias=b0_s[:, 2:3])
                c0 = cp.tile([128, B], F32, tag="c0", name="c0")
                if t == 0:
                    nc.vector.tensor_mul(c0[:], zi[:], zg[:])
                else:
                    ca = zp.tile([128, B], F32, tag="ca", name="ca")
                    cb = zp.tile([128, B], F32, tag="cb", name="cb")
                    nc.vector.tensor_mul(ca[:], zf[:], c0_prev[:])
                    nc.vector.tensor_mul(cb[:], zi[:], zg[:])
                    nc.vector.tensor_add(c0[:], ca[:], cb[:])
                c0_prev = c0
                tc0 = zp.tile([128, B], BF16, tag="tc0", name="tc0")
                nc.scalar.activation(tc0[:], c0[:], AF.Tanh)
                nc.scalar.activation(zo[:], ps0[3][:], AF.Sigmoid, bias=b0_s[:, 3:4])
                h0loc = zp.tile([128, B], F8, tag="h0loc", name="h0loc")
                nc.vector.tensor_mul(h0loc[:], zo[:], tc0[:])

                # -- AllGather h0(t): launch + copy-back on sync --
                ago0 = ag_launch(h0loc, "ag0")
                h0T_prev = h0T
                h0T = ag_copyback(ago0, "ag0", nc.sync)

                # -- copy-back of the h1 AllGather launched LAST iteration
                #    (its CC fired mid-previous-period: scalar never blocks) --
                if ago1_pend is not None:
                    h1T = ag_copyback(ago1_pend, "ag1", nc.scalar)
                    ago1_pend = None

                # -- layer-1 of step t-1: W1x start-phase (reads h0T(t-1),
                #    a full period old -- no copy-back pacing), W1h stop-phase
                #    (reads h1T(t-2)), cell1, acc, then launch its AllGather --
                if t > 0:
                    ps1_prev = [
                        pp.tile([128, B], F32, tag=f"ps1{g}", name=f"ps1{g}") for g in range(4)
                    ]
                    for ms in (M_ORDER[:3], M_ORDER[3:]):
                        for k in range(KH):
                            for m in ms:
                                nc.tensor.matmul(
                                    ps1_prev[m][:],
                                    wh_lhsT(w1x_s, k, m),
                                    h0T_prev[:, k * B : (k + 1) * B],
                                    start=(k == 0),
                                    stop=(t - 1 == 0 and k == KH - 1),
                                )
                    if t - 1 > 0:
                        for ms in (M_ORDER[:3], M_ORDER[3:]):
                            for k in range(KH):
                                for m in ms:
                                    nc.tensor.matmul(
                                        ps1_prev[m][:],
                                        wh_lhsT(w1h_s, k, m),
                                        h1T[:, k * B : (k + 1) * B],
                                        start=False,
                                        stop=(k == KH - 1),
                                    )
                    yo = zp.tile([128, B], BF16, tag="yo", name="yo")
                    tc1 = emit_cell1(t - 1, ps1_prev, yo)
                    h1loc = zp.tile([128, B], F8, tag="h1loc", name="h1loc")
                    nc.vector.tensor_mul(h1loc[:], yo[:], tc1[:])
                    ago1_pend = ag_launch(h1loc, "ag1")

                # -- hoisted x-projection for step t+1 --
                if t + 1 < t_steps:
                    ps0_next = [
                        pp.tile([128, B], F32, tag=f"ps0{g}", name=f"ps0{g}") for g in range(4)
                    ]
                    emit_wx(ps0_next, xt_nxt, t + 1)

                if t + 1 < t_steps:
                    ps0 = ps0_next
                    xt_cur, xt_nxt = xt_nxt, xt_nxt2

            # -- epilogue: finish layer-1 of the last step --
            tl = t_steps - 1
            if ago1_pend is not None:
                h1T = ag_copyback(ago1_pend, "ag1", nc.scalar)
                ago1_pend = None
            ps1_prev = [
                pp.tile([128, B], F32, tag=f"ps1{g}", name=f"ps1{g}") for g in range(4)
            ]
            for ms in (M_ORDER[:3], M_ORDER[3:]):
                for k in range(KH):
                    for m in ms:
                        nc.tensor.matmul(
                            ps1_prev[m][:],
                            wh_lhsT(w1x_s, k, m),
                            h0T[:, k * B : (k + 1) * B],
                            start=(k == 0),
                            stop=(tl == 0 and k == KH - 1),
                        )
            if tl > 0:
                for ms in (M_ORDER[:3], M_ORDER[3:]):
                    for k in range(KH):
                        for m in ms:
                            nc.tensor.matmul(
                                ps1_prev[m][:],
                                wh_lhsT(w1h_s, k, m),
                                h1T[:, k * B : (k + 1) * B],
                                start=False,
                                stop=(k == KH - 1),
                            )
            yo_l = zp.tile([128, B], BF16, tag="yo", name="yo")
            emit_cell1(tl, ps1_prev, yo_l)

            # -- decoder partial: out_p = (acc/T) . wdec (host sums cores) --
            psd = pp.tile([128, B], F32, tag="ps10", name="ps10")
            nc.tensor.matmul(
                psd[0:1, :], wdec_s[:, 0:1], acc[:],
                start=True, stop=True,
            )
            outt = zp.tile([1, B], F32, tag="outt", name="outt")
            nc.scalar.copy(outt[:], psd[0:1, :])
            nc.sync.dma_start(out_p.ap(), outt[:])

    nc.compile()
    return nc


def _prep_inputs(x, W_ih0, W_hh0, b_ih0, b_hh0, W_ih1, W_hh1, b_ih1, b_hh1, W_dec, t_steps, t_total=None):
    import ml_dtypes

    bf16 = ml_dtypes.bfloat16
    f8 = ml_dtypes.float8_e4m3
    t_total = t_total or t_steps
    xT = np.ascontiguousarray(np.transpose(x[:, :t_total, :], (1, 2, 0))).astype(bf16)
    b0 = (b_ih0 + b_hh0).astype(np.float32)
    b1 = (b_ih1 + b_hh1).astype(np.float32)
    in_maps = []
    for c in range(NCORES):
        rows = np.concatenate([g * H + np.arange(c * HL, (c + 1) * HL) for g in range(4)])

        def pack(W, nk, dt=bf16):
            Wt = np.ascontiguousarray(W[rows, :].T.astype(np.float32))  # [K_total, GL]
            arr = np.zeros((128, nk * GL), np.float32)
            for k in range(nk):
                kp = min(128, Wt.shape[0] - k * 128)
                arr[0:kp, k * GL : k * GL + GL] = Wt[k * 128 : k * 128 + kp, :]
            return arr.astype(dt)

        in_maps.append({
            "xT": xT,
            "w0x": pack(W_ih0, 3),
            "w0h": pack(W_hh0, KH),
            "w1x": pack(W_ih1, KH),
            "w1h": pack(W_hh1, KH),
            "b0d": np.ascontiguousarray(b0[rows].reshape(4, HL).T),
            "b1d": np.ascontiguousarray(b1[rows].reshape(4, HL).T),
            "wdec": np.ascontiguousarray(
                (W_dec[0, c * HL : (c + 1) * HL] / np.float32(t_steps)).reshape(HL, 1)
            ).astype(np.float32),
        })
    return in_maps


def _run(inputs, t_steps, **spmd_kwargs):
    nc = _build(t_steps)
    in_maps = _prep_inputs(
        inputs["x"], inputs["W_ih0"], inputs["W_hh0"], inputs["b_ih0"], inputs["b_hh0"],
        inputs["W_ih1"], inputs["W_hh1"], inputs["b_ih1"], inputs["b_hh1"], inputs["W_dec"],
        t_steps,
    )
    res = run_bass_kernel_spmd(nc, in_maps, core_ids=list(range(NCORES)), **spmd_kwargs)
    part = sum(res.results[c]["out_p"][0] for c in range(NCORES))
    out = (part + inputs["b_dec"][0]).astype(np.float32).reshape(B, 1)
    return out, res


def kernel(**inputs):
    out, _ = _run(inputs, T)
    return out
